# revision 1
# baseline (speedup 1.0000x reference)
"""Trainium2 Bass kernel for BeliefPlausibility (Dempster-Shafer bel/pl maps).

Problem: input [4, 384, 1248, 7] fp32 (6 singleton masses + omega per pixel).
Output: tuple (bel, pl), each [4, 384, 1248, 64] fp32 where, per pixel with
masses m_0..m_5 and omega w:
    bel[q] = sum_c m_c * ((q >> c) & 1)  for q in 1..62;  bel[0]=0, bel[63]=1
    pl[q]  = bel[q] + w                  for q in 1..62;  pl[0]=0,  pl[63]=1

Strategy (pure data parallel over 8 cores, no cross-core communication):
  - Flatten pixels; each core gets 239,616 pixels as [117, 128, 112]
    (117 supertiles x 128 partitions x 16 pixels x 7 channels).
  - Per supertile: contiguous DMA in [128, 112]; PE-transpose to channels-on-
    partitions; two fp32 matmuls against a constant [112, 1024] membership
    matrix produce PSUM [128, 512] already in the per-pixel bel layout
    (8 pixel-groups x 64 output columns); ACT copies bel PSUM->SBUF; DVE
    derives pl = bel + omega with a zero-stride broadcast AP; constant
    columns 0/63 are written directly; two contiguous 512 KB DMAs per
    output store the results.
  - Walrus allows only ONE sync-wait on an fp32 (self-weight-loading)
    Matmult, so tiny "absorber" matmuls (d1/d2 into a dummy PSUM tile)
    observe the in-DMA / DVE ticks first, keeping every real Matmult at
    <=1 wait.  PSUM is read by a single engine per tensor (ACT for the
    matmul banks, DVE for the transpose bank) for the same reason.
"""

import os
import sys

import numpy as np

if "concourse" not in sys.modules:
    try:
        import concourse  # noqa: F401
    except ImportError:
        sys.path.insert(0, "/opt/trn_rl_repo")

import concourse.bacc as bacc
import concourse.bass as bass
import concourse.mybir as mybir
import concourse.tile as tile
from concourse.bass_utils import run_bass_kernel_spmd

F32 = mybir.dt.float32
F32R = mybir.dt.float32r
ACT_COPY = mybir.ActivationFunctionType.Copy

N_CORES = 8
PX_TOTAL = 4 * 384 * 1248          # 1,916,928 pixels
PX_CORE = PX_TOTAL // N_CORES      # 239,616
PX_PART = 16                       # pixels per partition per supertile
PX_TILE = 128 * PX_PART            # 2048 pixels per supertile
N_TILES = PX_CORE // PX_TILE       # 117
N_CH = 7                           # 6 singletons + omega
N_SUB = 64                         # output positions per pixel
K_ROWS = PX_PART * N_CH            # 112 channel rows
GROUPS_PER_MM = 8                  # pixel-groups covered by one matmul
N_MM = PX_PART // GROUPS_PER_MM    # 2 matmuls per supertile


def _weight_matrix() -> np.ndarray:
    """[112, 1024]: W[7j+c, 512h+64g+q] = (q>>c)&1 for j=8h+g, q in 1..62,
    c in 0..5.  Columns (g,0) and (g,63) stay zero (written separately)."""
    w = np.zeros((K_ROWS, N_MM * 512), np.float32)
    for h in range(N_MM):
        for g in range(GROUPS_PER_MM):
            j = GROUPS_PER_MM * h + g
            col0 = 512 * h + 64 * g
            for q in range(1, 63):
                for c in range(6):
                    if (q >> c) & 1:
                        w[7 * j + c, col0 + q] = 1.0
    return w


def build_program(n_tiles: int = N_TILES, reps: int = 1,
                  use_f32r: bool = False, skip_pl: bool = False,
                  skip_out: bool = False, skip_mm: bool = False) -> bass.Bass:
    # Bacc (not plain Bass): its compile() runs generate_event_semaphores,
    # which splits multi-semaphore waits into standalone event-sem
    # instructions (TRN2 allows at most one wait per instruction).
    nc = bacc.Bacc("TRN2")

    x = nc.dram_tensor("x", (n_tiles, 128, PX_PART * N_CH), F32,
                       kind="ExternalInput")
    bel = nc.dram_tensor("bel", (n_tiles, 128, PX_PART * N_SUB), F32,
                         kind="ExternalOutput")
    pl = nc.dram_tensor("pl", (n_tiles, 128, PX_PART * N_SUB), F32,
                        kind="ExternalOutput")

    w_dram = nc.inline_tensor(_weight_matrix(), name="wmat")
    id_dram = nc.inline_tensor(np.eye(128, dtype=np.float32), name="ident")

    with tile.TileContext(nc) as tc:
        with (
            tc.tile_pool(name="const", bufs=1) as cpool,
            tc.tile_pool(name="inp", bufs=8) as inpool,
            tc.tile_pool(name="tp", bufs=4) as tpool,
            tc.tile_pool(name="om", bufs=4) as ompool,
            tc.tile_pool(name="outb", bufs=4) as belpool,
            tc.tile_pool(name="outp", bufs=4) as plpool,
            tc.tile_pool(name="psT", bufs=3, space="PSUM") as psTpool,
            tc.tile_pool(name="psM", bufs=1, space="PSUM") as psMpool,
            tc.tile_pool(name="psD", bufs=1, space="PSUM") as psDpool,
        ):
            # Stage the constants through an ACT copy: matmuls reading an
            # ACT-produced tensor can merge that dep with their other ACT
            # deps into a single semaphore wait (walrus allows only one
            # sync-wait on fp32 Matmults).
            mm_dt = F32R if use_f32r else F32
            wstage = cpool.tile([K_ROWS, N_MM * 512], F32)
            nc.sync.dma_start(wstage[:], w_dram[:])
            wmat = cpool.tile([K_ROWS, N_MM * 512], mm_dt)
            nc.scalar.copy(wmat[:], wstage[:])
            istage = cpool.tile([128, 128], F32)
            nc.sync.dma_start(istage[:], id_dram[:])
            ident = cpool.tile([128, 128], F32)
            nc.scalar.copy(ident[:], istage[:])
            dum = psDpool.tile([1, 1], F32)
            # One persistent 4-bank PSUM tensor, slices cycled manually:
            # avoids pool-release machinery so matmul slot-reuse deps stay
            # byte-range (same-engine WAW = program order, reader WAR = ACT).
            ps_all = psMpool.tile([128, 4 * 512], F32)

            for g in range(reps * n_tiles):
                t = g % n_tiles
                in_tile = inpool.tile([128, K_ROWS], F32)
                nc.sync.dma_start(in_tile[:], x[t])

                # d1: absorb the in-DMA wait on PE so the transpose
                # (an fp32 Matmult, max one sync-wait) stays at <=1 wait.
                nc.tensor.matmul(dum[:], in_tile[0:1, 0:1], in_tile[0:1, 0:1])

                ps_t = psTpool.tile([K_ROWS, 128], F32)
                nc.tensor.transpose(ps_t[:], in_tile[:], ident[:])

                # `that` is produced on ACT so the matmuls' two deps (data
                # RAW + PSUM-slot release, whose reader is also ACT) merge
                # into a single ACT semaphore wait.
                that = tpool.tile([K_ROWS, 128], mm_dt)
                nc.scalar.copy(that[:], ps_t[:])

                # Stage the omega channels through DVE: the pl tensor_add
                # then reads only DVE- and ACT-produced operands, keeping
                # it at one semaphore wait (ISA limit per instruction).
                omg = ompool.tile([128, PX_PART], F32)
                nc.vector.tensor_copy(omg[:], in_tile[:, 6:K_ROWS:7])

                bel_t = belpool.tile([128, PX_PART * N_SUB], F32)
                pl_t = plpool.tile([128, PX_PART * N_SUB], F32)
                bel3 = bel_t[:].rearrange("p (g q) -> p g q", q=N_SUB)
                pl3 = pl_t[:].rearrange("p (g q) -> p g q", q=N_SUB)

                # constant columns: bel/pl col 63 = 1, pl col 0 = 0
                # (bel col 0 comes from the all-zero W column via the copy).
                nc.scalar.activation(bel3[:, :, 63:64], ident[:, 0:PX_PART],
                                     ACT_COPY, bias=1.0, scale=0.0)
                if not skip_pl:
                    nc.vector.memset(pl3[:, :, 0:1], 0.0)
                    nc.vector.memset(pl3[:, :, 63:64], 1.0)

                for h in range(N_MM):
                    if skip_mm:
                        break
                    slot = (2 * g + h) % 4
                    ps = ps_all[:, 512 * slot:512 * (slot + 1)]
                    nc.tensor.matmul(ps, that[:],
                                     wmat[:, 512 * h:512 * (h + 1)])
                    ps3 = ps.rearrange("p (g q) -> p g q", q=N_SUB)
                    gsl = slice(GROUPS_PER_MM * h, GROUPS_PER_MM * (h + 1))

                    # bel columns 0..62 of each group: ACT copy PSUM->SBUF
                    nc.scalar.copy(bel3[:, gsl, 0:63], ps3[:, :, 0:63])

                    if not skip_pl:
                        # pl cols 1..62: bel + omega (zero-stride broadcast)
                        om = omg[:, GROUPS_PER_MM * h:GROUPS_PER_MM * (h + 1)]
                        om = bass.AP(om.tensor, om.offset, om.ap + [[0, 62]])
                        nc.vector.tensor_add(pl3[:, gsl, 1:63],
                                             bel3[:, gsl, 1:63], om)

                if not skip_out:
                    nc.sync.dma_start(bel[t], bel_t[:])
                    if not skip_pl:
                        nc.sync.dma_start(pl[t], pl_t[:])

    nc.compile()
    return nc


_NC_CACHE: dict[int, bass.Bass] = {}


def _get_program(n_tiles: int) -> bass.Bass:
    if n_tiles not in _NC_CACHE:
        _NC_CACHE[n_tiles] = build_program(n_tiles)
    return _NC_CACHE[n_tiles]


def run_on_cores(x_flat: np.ndarray, **run_kwargs):
    """x_flat: [PX_TOTAL, 7] fp32. Returns (bel, pl) each [PX_TOTAL, 64],
    plus the raw BassKernelResults as third element."""
    nc = _get_program(N_TILES)
    in_maps = []
    for c in range(N_CORES):
        shard = np.ascontiguousarray(
            x_flat[c * PX_CORE:(c + 1) * PX_CORE]).reshape(
                N_TILES, 128, PX_PART * N_CH)
        in_maps.append({"x": shard})
    rr = run_bass_kernel_spmd(nc, in_maps, core_ids=list(range(N_CORES)),
                              **run_kwargs)
    bel = np.empty((PX_TOTAL, N_SUB), np.float32)
    pl = np.empty((PX_TOTAL, N_SUB), np.float32)
    for c, res in enumerate(rr.results):
        sl = slice(c * PX_CORE, (c + 1) * PX_CORE)
        bel[sl] = np.asarray(res["bel"]).reshape(PX_CORE, N_SUB)
        pl[sl] = np.asarray(res["pl"]).reshape(PX_CORE, N_SUB)
    return bel, pl, rr


def kernel(inputs: np.ndarray):
    inputs = np.ascontiguousarray(np.asarray(inputs, dtype=np.float32))
    b, hh, ww, ch = inputs.shape
    x_flat = inputs.reshape(-1, ch)
    bel, pl, _ = run_on_cores(x_flat)
    return (bel.reshape(b, hh, ww, N_SUB), pl.reshape(b, hh, ww, N_SUB))



# revision 5
# speedup vs baseline: 1.4343x; 1.4343x over previous
"""Trainium2 Bass kernel for BeliefPlausibility (Dempster-Shafer bel/pl maps).

Problem: input [4, 384, 1248, 7] fp32 (6 singleton masses + omega per pixel).
Output: tuple (bel, pl), each [4, 384, 1248, 64] fp32 where, per pixel with
masses m_0..m_5 and omega w:
    bel[q] = sum_c m_c * ((q >> c) & 1)  for q in 1..62;  bel[0]=0, bel[63]=1
    pl[q]  = bel[q] + w                  for q in 1..62;  pl[0]=0,  pl[63]=1

Strategy (pure data parallel over 8 cores, no cross-core communication):
  - The kernel is memory-bound: outputs are 2 x 64 channels vs 7 input
    channels.  Everything runs in bf16 (inputs host-cast, outputs
    host-upcast); the 2e-2 relative-error budget dwarfs bf16's 2^-9
    rounding, and halving the output bytes halves the HBM-write floor.
  - Each core gets 239,616 pixels.  The host pre-permutes its shard to
    lhsT layout [112, 117*128]: row 7j+c = channel c of pixel-group j,
    column t*128+blk = pixel block.  The whole shard (30 KB/partition)
    is DMA'd into SBUF once and sliced per supertile -- no PE transpose,
    no per-tile input DMA.
  - Per supertile t (117 of them, 2048 pixels each): two bf16 matmuls
    [112,128] x [112,512] -> PSUM [128, 2x512] give bel for 16 pixel
    groups x 64 subsets, accumulated exactly in fp32.  The weight
    matrix also routes omega into column 63 of each group, so the DVE
    derives pl = bel + omega with a zero-stride broadcast straight from
    PSUM and writes bf16 to SBUF; ACT casts the bel columns to bf16.
  - bel/pl SBUF tiles are 4 persistent slots; constant columns (bel/pl
    0 and 63) are written once per slot before the loop, keeping the
    per-tile path to 2 matmuls + 1 ACT copy + 1 DVE add + 2 x 256 KB
    contiguous output DMAs.
"""

import sys

if "concourse" not in sys.modules:
    try:
        import concourse  # noqa: F401
    except ImportError:
        sys.path.insert(0, "/opt/trn_rl_repo")

import ml_dtypes
import numpy as np

import concourse.bacc as bacc
import concourse.bass as bass
import concourse.mybir as mybir
import concourse.tile as tile
from concourse.bass_utils import run_bass_kernel_spmd

F32 = mybir.dt.float32
BF16 = mybir.dt.bfloat16

N_CORES = 8
PX_TOTAL = 4 * 384 * 1248          # 1,916,928 pixels
PX_CORE = PX_TOTAL // N_CORES      # 239,616
PX_PART = 16                       # pixel groups per block (partition)
PX_TILE = 128 * PX_PART            # 2048 pixels per supertile
N_TILES = PX_CORE // PX_TILE       # 117
N_CH = 7                           # 6 singletons + omega
N_SUB = 64                         # output positions per pixel
K_ROWS = PX_PART * N_CH            # 112 contraction rows
GROUPS_PER_MM = 8                  # pixel-groups covered by one matmul
N_MM = PX_PART // GROUPS_PER_MM    # 2 matmuls per supertile
N_SLOTS = 4                        # bel/pl SBUF slots & PSUM banks


def _weight_matrix() -> np.ndarray:
    """[112, 1024]: W[7j+c, 512h+64g+q] = (q>>c)&1 for j=8h+g, q in 1..62,
    c in 0..5; W[7j+6, 512h+64g+63] = 1 (omega lane for the pl broadcast).
    Columns (g,0) stay zero; bel/pl column 63 is fixed up on-chip."""
    w = np.zeros((K_ROWS, N_MM * 512), np.float32)
    for h in range(N_MM):
        for g in range(GROUPS_PER_MM):
            j = GROUPS_PER_MM * h + g
            col0 = 512 * h + 64 * g
            for q in range(1, 63):
                for c in range(6):
                    if (q >> c) & 1:
                        w[7 * j + c, col0 + q] = 1.0
            w[7 * j + 6, col0 + 63] = 1.0
    return w


def build_program(n_tiles: int = N_TILES, reps: int = 1,
                  omega_via: str = "act_sbuf") -> bass.Bass:
    # Bacc (not plain Bass): its compile() runs generate_event_semaphores,
    # which splits multi-semaphore waits into standalone event-sem
    # instructions (TRN2 allows at most one wait per instruction).
    nc = bacc.Bacc("TRN2")

    x = nc.dram_tensor("x", (K_ROWS, n_tiles * 128), BF16,
                       kind="ExternalInput")
    bel = nc.dram_tensor("bel", (n_tiles, 128, PX_PART * N_SUB), BF16,
                         kind="ExternalOutput")
    pl = nc.dram_tensor("pl", (n_tiles, 128, PX_PART * N_SUB), BF16,
                        kind="ExternalOutput")

    w_dram = nc.inline_tensor(
        _weight_matrix().astype(ml_dtypes.bfloat16), name="wmat")

    with tile.TileContext(nc) as tc:
        with (
            tc.tile_pool(name="const", bufs=1) as cpool,
            tc.tile_pool(name="outb", bufs=1) as belpool,
            tc.tile_pool(name="outp", bufs=1) as plpool,
            tc.tile_pool(name="om", bufs=4) as ompool,
            tc.tile_pool(name="psM", bufs=1, space="PSUM") as psMpool,
        ):
            wmat = cpool.tile([K_ROWS, N_MM * 512], BF16)
            nc.sync.dma_start(wmat[:], w_dram[:])
            x_all = cpool.tile([K_ROWS, n_tiles * 128], BF16)
            nc.sync.dma_start(x_all[:], x[:])

            # Persistent slot-cycled tensors: PSUM banks for the matmuls
            # plus bel/pl staging tiles whose constant columns (0 and 63)
            # are written once, off the per-tile path.
            ps_all = psMpool.tile([128, N_SLOTS * 512], F32)
            bel_all = belpool.tile([128, N_SLOTS * PX_PART * N_SUB], BF16)
            pl_all = plpool.tile([128, N_SLOTS * PX_PART * N_SUB], BF16)
            bel4 = bel_all[:].rearrange("p (s g q) -> p s g q",
                                        s=N_SLOTS, q=N_SUB)
            pl4 = pl_all[:].rearrange("p (s g q) -> p s g q",
                                      s=N_SLOTS, q=N_SUB)
            for s in range(N_SLOTS):
                nc.vector.memset(bel4[:, s, :, 63:64], 1.0)
                nc.vector.memset(pl4[:, s, :, 0:1], 0.0)
                nc.vector.memset(pl4[:, s, :, 63:64], 1.0)

            stride = PX_PART * N_SUB
            for it in range(reps * n_tiles):
                t = it % n_tiles
                slot = it % N_SLOTS
                lhsT = x_all[:, t * 128:(t + 1) * 128]
                bel_t = bel_all[:, slot * stride:(slot + 1) * stride]
                pl_t = pl_all[:, slot * stride:(slot + 1) * stride]
                bel3 = bel_t.rearrange("p (g q) -> p g q", q=N_SUB)
                pl3 = pl_t.rearrange("p (g q) -> p g q", q=N_SUB)

                if omega_via != "dve_psum":
                    omg = ompool.tile([128, PX_PART],
                                      F32 if omega_via == "act_sbuf" else BF16)
                for h in range(N_MM):
                    bank = (N_MM * it + h) % N_SLOTS
                    ps = ps_all[:, 512 * bank:512 * (bank + 1)]
                    nc.tensor.matmul(ps, lhsT,
                                     wmat[:, 512 * h:512 * (h + 1)])
                    ps3 = ps.rearrange("p (g q) -> p g q", q=N_SUB)
                    gsl = slice(GROUPS_PER_MM * h, GROUPS_PER_MM * (h + 1))

                    # bel columns 0..62 of each group: ACT casts PSUM->bf16
                    nc.scalar.copy(bel3[:, gsl, 0:63], ps3[:, :, 0:63])

                    # pl cols 1..62: bel + omega (the weight matrix routed
                    # omega into PSUM column 63 of each group)
                    if omega_via == "dve_psum":
                        om = ps3[:, :, 63:64]
                        om = bass.AP(om.tensor, om.offset,
                                     om.ap[:-1] + [[0, 62]])
                        nc.vector.tensor_add(pl3[:, gsl, 1:63],
                                             ps3[:, :, 1:63], om)
                    elif omega_via == "act_sbuf":
                        # stage omega PSUM->SBUF on ACT; DVE add reads one
                        # PSUM operand (bel) + one SBUF operand (omega)
                        omh = omg[:, gsl]
                        nc.scalar.copy(omh, ps[:, 63:512:64])
                        om = bass.AP(omh.tensor, omh.offset,
                                     omh.ap + [[0, 62]])
                        nc.vector.tensor_add(pl3[:, gsl, 1:63],
                                             ps3[:, :, 1:63], om)
                    else:  # "sbuf_bf16": all-SBUF bf16 DVE add
                        omh = omg[:, gsl]
                        nc.scalar.copy(omh, ps[:, 63:512:64])
                        om = bass.AP(omh.tensor, omh.offset,
                                     omh.ap + [[0, 62]])
                        nc.vector.tensor_add(pl3[:, gsl, 1:63],
                                             bel3[:, gsl, 1:63], om)

                nc.sync.dma_start(bel[t], bel_t)
                nc.sync.dma_start(pl[t], pl_t)

    nc.compile()
    return nc


_NC_CACHE: dict[int, bass.Bass] = {}


def _get_program(n_tiles: int) -> bass.Bass:
    if n_tiles not in _NC_CACHE:
        _NC_CACHE[n_tiles] = build_program(n_tiles)
    return _NC_CACHE[n_tiles]


def run_on_cores(x_flat: np.ndarray, **run_kwargs):
    """x_flat: [PX_TOTAL, 7] fp32. Returns (bel, pl) each [PX_TOTAL, 64]
    fp32, plus the raw BassKernelResults as third element."""
    nc = _get_program(N_TILES)
    in_maps = []
    for c in range(N_CORES):
        seg = x_flat[c * PX_CORE:(c + 1) * PX_CORE]
        # [t, blk, j, c] -> rows (j, c), cols (t, blk): lhsT layout
        x4 = seg.reshape(N_TILES, 128, PX_PART, N_CH)
        xp = x4.transpose(2, 3, 0, 1).reshape(K_ROWS, N_TILES * 128)
        in_maps.append({"x": np.ascontiguousarray(
            xp.astype(ml_dtypes.bfloat16))})
    rr = run_bass_kernel_spmd(nc, in_maps, core_ids=list(range(N_CORES)),
                              **run_kwargs)
    bel = np.empty((PX_TOTAL, N_SUB), np.float32)
    pl = np.empty((PX_TOTAL, N_SUB), np.float32)
    for c, res in enumerate(rr.results):
        sl = slice(c * PX_CORE, (c + 1) * PX_CORE)
        bel[sl] = np.asarray(res["bel"]).astype(np.float32).reshape(
            PX_CORE, N_SUB)
        pl[sl] = np.asarray(res["pl"]).astype(np.float32).reshape(
            PX_CORE, N_SUB)
    return bel, pl, rr


def kernel(inputs: np.ndarray):
    inputs = np.ascontiguousarray(np.asarray(inputs, dtype=np.float32))
    b, hh, ww, ch = inputs.shape
    x_flat = inputs.reshape(-1, ch)
    bel, pl, _ = run_on_cores(x_flat)
    return (bel.reshape(b, hh, ww, N_SUB), pl.reshape(b, hh, ww, N_SUB))


# revision 8
# speedup vs baseline: 4.4055x; 3.0715x over previous
"""Trainium2 Bass kernel for BeliefPlausibility (Dempster-Shafer bel/pl maps).

Problem: input [4, 384, 1248, 7] fp32 (6 singleton masses + omega per pixel).
Output: tuple (bel, pl), each [4, 384, 1248, 64] fp32 where, per pixel with
masses m_0..m_5 and omega w:
    bel[q] = sum_c m_c * ((q >> c) & 1)  for q in 1..62;  bel[0]=0, bel[63]=1
    pl[q]  = bel[q] + w                  for q in 1..62;  pl[0]=0,  pl[63]=1

Strategy (pure data parallel over 8 cores, no cross-core communication):
  - The kernel is memory-bound: outputs are 2 x 64 channels vs 7 input
    channels.  Everything runs in bf16 (inputs host-cast, outputs
    host-upcast); the 2e-2 relative-error budget dwarfs bf16's 2^-9
    rounding, and halving the output bytes halves the HBM-write floor.
  - Each core gets 239,616 pixels.  The host pre-permutes its shard to
    lhsT layout [112, 117*128]: row 7j+c = channel c of pixel-group j,
    column t*128+blk = pixel block.  The whole shard (30 KB/partition)
    is DMA'd into SBUF once and sliced per supertile -- no PE transpose,
    no per-tile input DMA.
  - Per supertile t (117 of them, 2048 pixels each): bf16 matmul(s)
    [112,128] x [112,1024] -> one PSUM bank pair [128, 1024] give bel
    for 16 pixel groups x 64 subsets, accumulated exactly in fp32.  The
    weight matrix also routes omega into column 63 of each group.  One
    ACT copy casts bel columns 0..62 PSUM->SBUF bf16; one DVE add forms
    pl = bel + omega (omega broadcast straight from PSUM column 63 with
    a zero-stride AP) writing bf16.
  - bel/pl SBUF staging is 3 persistent buffers x 3 supertiles; the
    constant columns (bel/pl 0 and 63) are written once per buffer
    before the loop, and each buffer drains with one contiguous 768 KB
    DMA per output tensor, keeping the loop at ~5 instructions/tile.
"""

import sys

if "concourse" not in sys.modules:
    try:
        import concourse  # noqa: F401
    except ImportError:
        sys.path.insert(0, "/opt/trn_rl_repo")

import ml_dtypes
import numpy as np

import concourse.bacc as bacc
import concourse.bass as bass
import concourse.mybir as mybir
import concourse.tile as tile
from concourse.bass_utils import run_bass_kernel_spmd

F32 = mybir.dt.float32
BF16 = mybir.dt.bfloat16

N_CORES = 8
PX_TOTAL = 4 * 384 * 1248          # 1,916,928 pixels
PX_CORE = PX_TOTAL // N_CORES      # 239,616
PX_PART = 16                       # pixel groups per block (partition)
PX_TILE = 128 * PX_PART            # 2048 pixels per supertile
N_TILES = PX_CORE // PX_TILE       # 117
N_CH = 7                           # 6 singletons + omega
N_SUB = 64                         # output positions per pixel
K_ROWS = PX_PART * N_CH            # 112 contraction rows
TILE_W = PX_PART * N_SUB           # 1024 outputs per partition per tile
N_PS = 4                           # PSUM bank-pair rotation depth
OUT_GRP = 3                        # supertiles per output staging buffer
N_OBUF = 3                         # output staging buffers (bel & pl each)


def _weight_matrix() -> np.ndarray:
    """[112, 1024]: W[7j+c, 64j+q] = (q>>c)&1 for q in 1..62, c in 0..5;
    W[7j+6, 64j+63] = 1 (omega lane for the pl broadcast).  Columns
    (j,0) stay zero; bel/pl column 63 is fixed up on-chip."""
    w = np.zeros((K_ROWS, TILE_W), np.float32)
    for j in range(PX_PART):
        for q in range(1, 63):
            for c in range(6):
                if (q >> c) & 1:
                    w[7 * j + c, 64 * j + q] = 1.0
        w[7 * j + 6, 64 * j + 63] = 1.0
    return w


def build_program(n_tiles: int = N_TILES, reps: int = 1,
                  wide_mm: bool = False, omega_via: str = "mix_psum",
                  out_grp: int = OUT_GRP) -> bass.Bass:
    # Bacc (not plain Bass): its compile() runs generate_event_semaphores,
    # which splits multi-semaphore waits into standalone event-sem
    # instructions (TRN2 allows at most one wait per instruction).
    assert n_tiles % out_grp == 0
    nc = bacc.Bacc("TRN2")

    x = nc.dram_tensor("x", (K_ROWS, n_tiles * 128), BF16,
                       kind="ExternalInput")
    bel = nc.dram_tensor("bel", (n_tiles, 128, TILE_W), BF16,
                         kind="ExternalOutput")
    pl = nc.dram_tensor("pl", (n_tiles, 128, TILE_W), BF16,
                        kind="ExternalOutput")

    w_dram = nc.inline_tensor(
        _weight_matrix().astype(ml_dtypes.bfloat16), name="wmat")

    with tile.TileContext(nc) as tc:
        with (
            tc.tile_pool(name="const", bufs=1) as cpool,
            tc.tile_pool(name="outb", bufs=1) as belpool,
            tc.tile_pool(name="outp", bufs=1) as plpool,
            tc.tile_pool(name="psM", bufs=1, space="PSUM") as psMpool,
        ):
            wmat = cpool.tile([K_ROWS, TILE_W], BF16)
            nc.sync.dma_start(wmat[:], w_dram[:])
            x_all = cpool.tile([K_ROWS, n_tiles * 128], BF16)
            nc.sync.dma_start(x_all[:], x[:])

            # Persistent slot-cycled tensors: PSUM bank pairs for the
            # matmuls, and bel/pl staging buffers of OUT_GRP supertiles
            # whose constant columns (0/63) are written once, pre-loop.
            ps_all = psMpool.tile([128, N_PS * TILE_W], F32)
            gw = out_grp * TILE_W
            bel_all = belpool.tile([128, N_OBUF * gw], BF16)
            pl_all = plpool.tile([128, N_OBUF * gw], BF16)
            bel4 = bel_all[:].rearrange("p (b g q) -> p b g q",
                                        b=N_OBUF, q=N_SUB)
            pl4 = pl_all[:].rearrange("p (b g q) -> p b g q",
                                      b=N_OBUF, q=N_SUB)
            for s in range(N_OBUF):
                nc.vector.memset(bel4[:, s, :, 63:64], 1.0)
                nc.vector.memset(pl4[:, s, :, 0:1], 0.0)
                nc.vector.memset(pl4[:, s, :, 63:64], 1.0)

            for it in range(reps * n_tiles):
                t = it % n_tiles
                grp, tt = divmod(t, out_grp)
                buf = grp % N_OBUF
                ps = ps_all[:, TILE_W * (it % N_PS):TILE_W * (it % N_PS + 1)]
                ps3 = ps.rearrange("p (g q) -> p g q", q=N_SUB)
                lhsT = x_all[:, t * 128:(t + 1) * 128]
                off = buf * gw + tt * TILE_W
                bel3 = bel_all[:, off:off + TILE_W].rearrange(
                    "p (g q) -> p g q", q=N_SUB)
                pl3 = pl_all[:, off:off + TILE_W].rearrange(
                    "p (g q) -> p g q", q=N_SUB)

                if wide_mm:
                    nc.tensor.matmul(ps, lhsT, wmat[:])
                else:
                    for h in range(2):
                        nc.tensor.matmul(ps[:, 512 * h:512 * (h + 1)], lhsT,
                                         wmat[:, 512 * h:512 * (h + 1)])

                # bel columns 0..62 of each group: ACT casts PSUM->bf16
                # (column 63 is the hoisted constant 1)
                nc.scalar.copy(bel3[:, :, 0:63], ps3[:, :, 0:63])

                # pl cols 1..62: bel + omega, omega broadcast straight
                # from PSUM column 63 via a zero-stride AP
                om = ps3[:, :, 63:64]
                om = bass.AP(om.tensor, om.offset, om.ap[:-1] + [[0, 62]])
                if omega_via == "mix_psum":
                    nc.vector.tensor_add(pl3[:, :, 1:63],
                                         bel3[:, :, 1:63], om)
                else:  # "psum_psum" would be illegal; fall back to staging
                    raise ValueError(omega_via)

                if tt == out_grp - 1:
                    # SBUF src stays partition-major; the DRAM dest AP is
                    # permuted to match its traversal order.
                    src_b = bel_all[:, buf * gw:(buf + 1) * gw].rearrange(
                        "p (s w) -> p s w", w=TILE_W)
                    src_p = pl_all[:, buf * gw:(buf + 1) * gw].rearrange(
                        "p (s w) -> p s w", w=TILE_W)
                    dst_b = bel[grp * out_grp:(grp + 1) * out_grp].rearrange(
                        "s p w -> p s w")
                    dst_p = pl[grp * out_grp:(grp + 1) * out_grp].rearrange(
                        "s p w -> p s w")
                    nc.sync.dma_start(dst_b, src_b)
                    nc.sync.dma_start(dst_p, src_p)

    nc.compile()
    return nc


_NC_CACHE: dict[int, bass.Bass] = {}


def _get_program(n_tiles: int) -> bass.Bass:
    if n_tiles not in _NC_CACHE:
        _NC_CACHE[n_tiles] = build_program(n_tiles)
    return _NC_CACHE[n_tiles]


def run_on_cores(x_flat: np.ndarray, **run_kwargs):
    """x_flat: [PX_TOTAL, 7] fp32. Returns (bel, pl) each [PX_TOTAL, 64]
    fp32, plus the raw BassKernelResults as third element."""
    nc = _get_program(N_TILES)
    in_maps = []
    for c in range(N_CORES):
        seg = x_flat[c * PX_CORE:(c + 1) * PX_CORE]
        # [t, blk, j, c] -> rows (j, c), cols (t, blk): lhsT layout
        x4 = seg.reshape(N_TILES, 128, PX_PART, N_CH)
        xp = x4.transpose(2, 3, 0, 1).reshape(K_ROWS, N_TILES * 128)
        in_maps.append({"x": np.ascontiguousarray(
            xp.astype(ml_dtypes.bfloat16))})
    rr = run_bass_kernel_spmd(nc, in_maps, core_ids=list(range(N_CORES)),
                              **run_kwargs)
    bel = np.empty((PX_TOTAL, N_SUB), np.float32)
    pl = np.empty((PX_TOTAL, N_SUB), np.float32)
    for c, res in enumerate(rr.results):
        sl = slice(c * PX_CORE, (c + 1) * PX_CORE)
        bel[sl] = np.asarray(res["bel"]).astype(np.float32).reshape(
            PX_CORE, N_SUB)
        pl[sl] = np.asarray(res["pl"]).astype(np.float32).reshape(
            PX_CORE, N_SUB)
    return bel, pl, rr


def kernel(inputs: np.ndarray):
    inputs = np.ascontiguousarray(np.asarray(inputs, dtype=np.float32))
    b, hh, ww, ch = inputs.shape
    x_flat = inputs.reshape(-1, ch)
    bel, pl, _ = run_on_cores(x_flat)
    return (bel.reshape(b, hh, ww, N_SUB), pl.reshape(b, hh, ww, N_SUB))


# revision 10
# speedup vs baseline: 4.6239x; 1.0496x over previous
"""Trainium2 Bass kernel for BeliefPlausibility (Dempster-Shafer bel/pl maps).

Problem: input [4, 384, 1248, 7] fp32 (6 singleton masses + omega per pixel).
Output: tuple (bel, pl), each [4, 384, 1248, 64] fp32 where, per pixel with
masses m_0..m_5 and omega w:
    bel[q] = sum_c m_c * ((q >> c) & 1)  for q in 1..62;  bel[0]=0, bel[63]=1
    pl[q]  = bel[q] + w                  for q in 1..62;  pl[0]=0,  pl[63]=1

Strategy (pure data parallel over 8 cores, no cross-core communication):
  - The kernel is memory-bound: outputs are 2 x 64 channels vs 7 input
    channels.  Everything runs in bf16 (inputs host-cast, outputs
    host-upcast); the 2e-2 relative-error budget dwarfs bf16's 2^-9
    rounding, and halving the output bytes halves the HBM-write floor.
  - Each core gets 239,616 pixels.  The host pre-permutes its shard to
    lhsT layout [112, 117*128]: row 7j+c = channel c of pixel-group j,
    column t*128+blk = pixel block.  The whole shard (30 KB/partition)
    is DMA'd into SBUF once and sliced per supertile -- no PE transpose,
    no per-tile input DMA.
  - Per supertile t (117 of them, 2048 pixels each): bf16 matmul(s)
    [112,128] x [112,1024] -> one PSUM bank pair [128, 1024] give bel
    for 16 pixel groups x 64 subsets, accumulated exactly in fp32.  The
    weight matrix also routes omega into column 63 of each group.  One
    ACT copy casts bel columns 0..62 PSUM->SBUF bf16; one DVE add forms
    pl = bel + omega (omega broadcast straight from PSUM column 63 with
    a zero-stride AP) writing bf16.
  - bel/pl SBUF staging is 3 persistent buffers x 3 supertiles; the
    constant columns (bel/pl 0 and 63) are written once per buffer
    before the loop, and each buffer drains with one contiguous 768 KB
    DMA per output tensor, keeping the loop at ~5 instructions/tile.
"""

import sys

if "concourse" not in sys.modules:
    try:
        import concourse  # noqa: F401
    except ImportError:
        sys.path.insert(0, "/opt/trn_rl_repo")

import ml_dtypes
import numpy as np

import concourse.bacc as bacc
import concourse.bass as bass
import concourse.mybir as mybir
import concourse.tile as tile
from concourse.bass_utils import run_bass_kernel_spmd

F32 = mybir.dt.float32
BF16 = mybir.dt.bfloat16

N_CORES = 8
PX_TOTAL = 4 * 384 * 1248          # 1,916,928 pixels
PX_CORE = PX_TOTAL // N_CORES      # 239,616
PX_PART = 16                       # pixel groups per block (partition)
PX_TILE = 128 * PX_PART            # 2048 pixels per supertile
N_TILES = PX_CORE // PX_TILE       # 117
N_CH = 7                           # 6 singletons + omega
N_SUB = 64                         # output positions per pixel
K_ROWS = PX_PART * N_CH            # 112 contraction rows
TILE_W = PX_PART * N_SUB           # 1024 outputs per partition per tile
N_PS = 4                           # PSUM bank-pair rotation depth
OUT_GRP = 3                        # supertiles per output staging buffer
N_OBUF = 3                         # output staging buffers (bel & pl each)


def _weight_matrix() -> np.ndarray:
    """[112, 1024]: W[7j+c, 64j+q] = (q>>c)&1 for q in 1..62, c in 0..5;
    W[7j+6, 64j+63] = 1 (omega lane for the pl broadcast).  Columns
    (j,0) stay zero; bel/pl column 63 is fixed up on-chip."""
    w = np.zeros((K_ROWS, TILE_W), np.float32)
    for j in range(PX_PART):
        for q in range(1, 63):
            for c in range(6):
                if (q >> c) & 1:
                    w[7 * j + c, 64 * j + q] = 1.0
        w[7 * j + 6, 64 * j + 63] = 1.0
    return w


def build_program(n_tiles: int = N_TILES, reps: int = 1,
                  wide_mm: bool = False, omega_via: str = "mix_psum",
                  out_grp: int = OUT_GRP) -> bass.Bass:
    # Bacc (not plain Bass): its compile() runs generate_event_semaphores,
    # which splits multi-semaphore waits into standalone event-sem
    # instructions (TRN2 allows at most one wait per instruction).
    assert n_tiles % out_grp == 0
    nc = bacc.Bacc("TRN2")

    x = nc.dram_tensor("x", (K_ROWS, n_tiles * 128), BF16,
                       kind="ExternalInput")
    bel = nc.dram_tensor("bel", (n_tiles, 128, TILE_W), BF16,
                         kind="ExternalOutput")
    pl = nc.dram_tensor("pl", (n_tiles, 128, TILE_W), BF16,
                        kind="ExternalOutput")

    w_dram = nc.inline_tensor(
        _weight_matrix().astype(ml_dtypes.bfloat16), name="wmat")

    with tile.TileContext(nc) as tc:
        with (
            tc.tile_pool(name="const", bufs=1) as cpool,
            tc.tile_pool(name="outb", bufs=1) as belpool,
            tc.tile_pool(name="outp", bufs=1) as plpool,
            tc.tile_pool(name="psM", bufs=1, space="PSUM") as psMpool,
        ):
            wmat = cpool.tile([K_ROWS, TILE_W], BF16)
            nc.sync.dma_start(wmat[:], w_dram[:])
            # Chunked input prefetch: the tile framework tracks byte-range
            # deps, so matmul t only waits for its own chunk and compute
            # starts ~1 chunk into the load instead of after all 3.35 MB.
            x_all = cpool.tile([K_ROWS, n_tiles * 128], BF16)
            n_ch_dma = 8
            ct = (n_tiles + n_ch_dma - 1) // n_ch_dma
            for k in range(0, n_tiles, ct):
                cols = slice(k * 128, min(n_tiles, k + ct) * 128)
                nc.sync.dma_start(x_all[:, cols], x[:, cols])

            # Persistent slot-cycled tensors: PSUM bank pairs for the
            # matmuls, and bel/pl staging buffers of OUT_GRP supertiles
            # whose constant columns (0/63) are written once, pre-loop.
            ps_all = psMpool.tile([128, N_PS * TILE_W], F32)
            gw = out_grp * TILE_W
            bel_all = belpool.tile([128, N_OBUF * gw], BF16)
            pl_all = plpool.tile([128, N_OBUF * gw], BF16)
            bel4 = bel_all[:].rearrange("p (b g q) -> p b g q",
                                        b=N_OBUF, q=N_SUB)
            pl4 = pl_all[:].rearrange("p (b g q) -> p b g q",
                                      b=N_OBUF, q=N_SUB)
            for s in range(N_OBUF):
                nc.vector.memset(bel4[:, s, :, 63:64], 1.0)
                nc.vector.memset(pl4[:, s, :, 0:1], 0.0)
                nc.vector.memset(pl4[:, s, :, 63:64], 1.0)

            for it in range(reps * n_tiles):
                t = it % n_tiles
                grp, tt = divmod(t, out_grp)
                buf = grp % N_OBUF
                ps = ps_all[:, TILE_W * (it % N_PS):TILE_W * (it % N_PS + 1)]
                ps3 = ps.rearrange("p (g q) -> p g q", q=N_SUB)
                lhsT = x_all[:, t * 128:(t + 1) * 128]
                off = buf * gw + tt * TILE_W
                bel3 = bel_all[:, off:off + TILE_W].rearrange(
                    "p (g q) -> p g q", q=N_SUB)
                pl3 = pl_all[:, off:off + TILE_W].rearrange(
                    "p (g q) -> p g q", q=N_SUB)

                if wide_mm:
                    nc.tensor.matmul(ps, lhsT, wmat[:])
                else:
                    for h in range(2):
                        nc.tensor.matmul(ps[:, 512 * h:512 * (h + 1)], lhsT,
                                         wmat[:, 512 * h:512 * (h + 1)])

                # bel columns 0..62 of each group: ACT casts PSUM->bf16
                # (column 63 is the hoisted constant 1)
                nc.scalar.copy(bel3[:, :, 0:63], ps3[:, :, 0:63])

                # pl cols 1..62: bel + omega, omega broadcast straight
                # from PSUM column 63 via a zero-stride AP
                om = ps3[:, :, 63:64]
                om = bass.AP(om.tensor, om.offset, om.ap[:-1] + [[0, 62]])
                if omega_via == "mix_psum":
                    nc.vector.tensor_add(pl3[:, :, 1:63],
                                         bel3[:, :, 1:63], om)
                else:  # "psum_psum" would be illegal; fall back to staging
                    raise ValueError(omega_via)

                last_grp = (t >= n_tiles - out_grp) and reps * n_tiles - it <= out_grp
                if last_grp:
                    # Final group drains per-tile so the tail DMA starts as
                    # soon as each tile's data is ready instead of after
                    # the whole group.
                    src_b = bel_all[:, off:off + TILE_W]
                    src_p = pl_all[:, off:off + TILE_W]
                    nc.sync.dma_start(bel[t], src_b)
                    nc.sync.dma_start(pl[t], src_p)
                elif tt == out_grp - 1:
                    # SBUF src stays partition-major; the DRAM dest AP is
                    # permuted to match its traversal order.
                    src_b = bel_all[:, buf * gw:(buf + 1) * gw].rearrange(
                        "p (s w) -> p s w", w=TILE_W)
                    src_p = pl_all[:, buf * gw:(buf + 1) * gw].rearrange(
                        "p (s w) -> p s w", w=TILE_W)
                    dst_b = bel[grp * out_grp:(grp + 1) * out_grp].rearrange(
                        "s p w -> p s w")
                    dst_p = pl[grp * out_grp:(grp + 1) * out_grp].rearrange(
                        "s p w -> p s w")
                    nc.sync.dma_start(dst_b, src_b)
                    nc.sync.dma_start(dst_p, src_p)

    nc.compile()
    return nc


_NC_CACHE: dict[int, bass.Bass] = {}


def _get_program(n_tiles: int) -> bass.Bass:
    if n_tiles not in _NC_CACHE:
        _NC_CACHE[n_tiles] = build_program(n_tiles)
    return _NC_CACHE[n_tiles]


def run_on_cores(x_flat: np.ndarray, **run_kwargs):
    """x_flat: [PX_TOTAL, 7] fp32. Returns (bel, pl) each [PX_TOTAL, 64]
    fp32, plus the raw BassKernelResults as third element."""
    nc = _get_program(N_TILES)
    in_maps = []
    for c in range(N_CORES):
        seg = x_flat[c * PX_CORE:(c + 1) * PX_CORE]
        # [t, blk, j, c] -> rows (j, c), cols (t, blk): lhsT layout
        x4 = seg.reshape(N_TILES, 128, PX_PART, N_CH)
        xp = x4.transpose(2, 3, 0, 1).reshape(K_ROWS, N_TILES * 128)
        in_maps.append({"x": np.ascontiguousarray(
            xp.astype(ml_dtypes.bfloat16))})
    rr = run_bass_kernel_spmd(nc, in_maps, core_ids=list(range(N_CORES)),
                              **run_kwargs)
    bel = np.empty((PX_TOTAL, N_SUB), np.float32)
    pl = np.empty((PX_TOTAL, N_SUB), np.float32)
    for c, res in enumerate(rr.results):
        sl = slice(c * PX_CORE, (c + 1) * PX_CORE)
        bel[sl] = np.asarray(res["bel"]).astype(np.float32).reshape(
            PX_CORE, N_SUB)
        pl[sl] = np.asarray(res["pl"]).astype(np.float32).reshape(
            PX_CORE, N_SUB)
    return bel, pl, rr


def kernel(inputs: np.ndarray):
    inputs = np.ascontiguousarray(np.asarray(inputs, dtype=np.float32))
    b, hh, ww, ch = inputs.shape
    x_flat = inputs.reshape(-1, ch)
    bel, pl, _ = run_on_cores(x_flat)
    return (bel.reshape(b, hh, ww, N_SUB), pl.reshape(b, hh, ww, N_SUB))
